# revision 1
# baseline (speedup 1.0000x reference)
"""DeepseekV2 MLA decode (matrix-absorbed) on 8 Trainium2 NeuronCores.

v4: bf16 streams, 3-queue split, big DMA tiles, overlapped output AllReduce.

Sharding:
  - W_DQ row-sharded (contraction) -> partial cQ -> AllReduce (49KB) -> RMSNorm
    computed redundantly on every core (ln_w folded into W_QR/W_UQ_UK host-side).
  - W_QR / W_UQ_UK head-sharded (16 of 128 heads per core).
  - AllGather of q (per-core [8,16,576] bf16).
  - Attention sharded over kv_len (1024 of 8192 positions per core, all 128
    heads); rope applied to k host-side with *relative* positions (q un-roped);
    softmax exp without max subtraction; partial (attn, lsum) -> ReduceScatter.
  - W_UV_O row-sharded (same 16 heads); final AllReduce of [8,5120] partials,
    split in column halves so the first AR overlaps the second half's matmuls.

Queues:
  - SP (sync) HWDGE: dep-free weight streams in priority order
    (wdq -> wqr -> wuk -> wuvo), large tiles only.
  - Act (scalar) HWDGE: kv-cache stream + the serial control path
    (collective bounce stores/loads); exp also runs on Act engine.
  - Pool (gpsimd): collectives only.
"""
import sys

if "/opt/trn_rl_repo" not in sys.path:
    sys.path.insert(0, "/opt/trn_rl_repo")

import numpy as np

N_CORES = 8
B = 8           # batch
H = 5120        # hidden
NH = 128        # heads
QLR = 1536      # q lora rank
ROPE = 64
KVLR = 512
KV = 8192
THETA = 10000.0
SCALE = 192.0 ** -0.5

HL = NH // N_CORES      # 16 local heads
KVL = KV // N_CORES     # 1024 local kv positions
HD = H // N_CORES       # 640 local hidden (stage-1 contraction shard)
KT = KVL // 128         # 8 kv tiles of 128 per core

_CACHE = {}


def build_nc(sim=False):
    import concourse.bacc as bacc
    import concourse.mybir as mybir
    import concourse.tile as tile

    F32 = mybir.dt.float32
    F32R = mybir.dt.float32r
    BF16 = mybir.dt.bfloat16
    AF = mybir.ActivationFunctionType

    nc = bacc.Bacc("TRN2", target_bir_lowering=False, debug=False,
                   num_devices=(1 if sim else N_CORES))

    # ---- per-core inputs ----
    hst = nc.dram_tensor("hst", [HD, B], BF16, kind="ExternalInput")
    wdq = nc.dram_tensor("wdq", [HD, QLR], BF16, kind="ExternalInput")
    wqr = nc.dram_tensor("wqr", [QLR, HL * ROPE], BF16, kind="ExternalInput")
    wuk = nc.dram_tensor("wuk", [QLR, HL * KVLR], BF16, kind="ExternalInput")
    ckv = nc.dram_tensor("ckv", [B, KVL, KVLR], BF16, kind="ExternalInput")
    ket = nc.dram_tensor("ket", [ROPE, B, KVL], BF16, kind="ExternalInput")
    identr = nc.dram_tensor("identr", [128, 128], F32R, kind="ExternalInput")
    identb = nc.dram_tensor("identb", [128, 128], BF16, kind="ExternalInput")
    wuvo = nc.dram_tensor("wuvo", [HL * KVLR, H], BF16, kind="ExternalInput")
    out = nc.dram_tensor("out", [B, H], F32, kind="ExternalOutput")

    RG = [list(range(N_CORES))]
    NHALF = H // 2

    def coll(kind, op, in_t, out_t):
        if not sim:
            nc.gpsimd.collective_compute(kind, op, replica_groups=RG,
                                         ins=[in_t.opt()], outs=[out_t.opt()])
        elif kind == "AllGather":
            nc.gpsimd.dma_start(out_t[0], in_t[:])
        elif kind == "ReduceScatter":
            nc.gpsimd.dma_start(out_t[:], in_t[0])
        else:
            nc.gpsimd.dma_start(out_t[:], in_t[:])

    with tile.TileContext(nc) as tc:
        with (
            tc.tile_pool(name="const", bufs=1) as cpool,
            tc.tile_pool(name="dram", bufs=1, space="DRAM") as dram,
            tc.tile_pool(name="wuvo_sb", bufs=8) as wvp,
            tc.tile_pool(name="ckv_sb", bufs=3) as ckp,
        ):
            idr = cpool.tile([128, 128], F32R)
            idb = cpool.tile([128, 128], BF16)
            eps = cpool.tile([8, 1], F32)
            nc.vector.memset(eps[:], 1e-6)
            ket_all = cpool.tile([ROPE, B, KVL], BF16)

            # collective bounce buffers
            cq_ar_in = dram.tile([B, QLR], F32)
            cq_ar_out = dram.tile([B, QLR], F32)
            q_ag_in = dram.tile([B, HL, KVLR + ROPE], BF16)
            q_ag_out = dram.tile([N_CORES, B, HL, KVLR + ROPE], BF16)
            at_rs_in = dram.tile([N_CORES, B, HL, KVLR + 1], BF16)
            at_rs_out = dram.tile([B, HL, KVLR + 1], BF16)
            o_ar_in = [dram.tile([B, NHALF], F32, name=f"o_ar_in{h}") for h in range(2)]
            o_ar_out = [dram.tile([B, NHALF], F32, name=f"o_ar_out{h}") for h in range(2)]

            def prefetch_ckv(b):
                t = ckp.tile([128, KT, KVLR], BF16, tag="ckv", name=f"ckv_t{b}")
                nc.scalar.dma_start(t[:], ckv[b].rearrange("(t p) l -> p t l", p=128))
                return t

            # =========== Stage 1: cQ = rmsnorm(hs @ W_DQ) ===========
            # stream pools are allocated BEFORE the stage-1 scratch pool so
            # they get fresh SBUF ranges: a pool that reuses a released zone
            # inherits a dependency on that zone's users, which would gate the
            # weight streams behind stage-1 compute.
            with (
                tc.tile_pool(name="s1b", bufs=1) as s1b,
                tc.tile_pool(name="wqr_sb", bufs=2) as wqp,
                tc.tile_pool(name="wuk_sb", bufs=5) as wkp,
            ):
                with tc.tile_pool(name="s1a", bufs=1) as s1a:
                    # SP queue: stage-1 weights first
                    hs_t = s1a.tile([128, 5, B], BF16)
                    nc.sync.dma_start(hs_t[:], hst[:].rearrange("(k p) b -> p k b", p=128))
                    nc.sync.dma_start(idr[:], identr[:])
                    nc.sync.dma_start(idb[:], identb[:])
                    # Act queue: cache prefetch (dep-free) ahead of control path
                    nc.scalar.dma_start(ket_all[:], ket[:])
                    ckv_tiles = {b: prefetch_ckv(b) for b in range(3)}

                    with tc.tile_pool(name="s1ps", bufs=1, space="PSUM") as s1ps:
                        cq_ps = s1ps.tile([8, QLR], F32)
                        for k in range(5):
                            wdq_t = s1a.tile([128, QLR], BF16, tag="wdq", bufs=2)
                            nc.sync.dma_start(
                                wdq_t[:],
                                wdq[:].rearrange("(k p) j -> p k j", p=128)[:, k, :])
                            for n in range(3):
                                nc.tensor.matmul(
                                    cq_ps[:, n * 512:(n + 1) * 512],
                                    hs_t[:, k, :],
                                    wdq_t[:, n * 512:(n + 1) * 512],
                                    start=(k == 0), stop=(k == 4),
                                )
                        cqraw = s1a.tile([8, QLR], F32)
                        nc.vector.tensor_copy(cqraw[:], cq_ps[:])
                    nc.scalar.dma_start(cq_ar_in[:], cqraw[:])
                    coll("AllReduce", mybir.AluOpType.add, cq_ar_in, cq_ar_out)
                    cqsum = s1a.tile([8, QLR], F32)
                    nc.scalar.dma_start(cqsum[:], cq_ar_out[:])
                    # rmsnorm (ln_w folded into weights host-side); Square
                    # output is scratch — reuse cqraw (its store has completed)
                    ssq = s1a.tile([8, 1], F32)
                    nc.scalar.activation(cqraw[:], cqsum[:], AF.Square, accum_out=ssq[:])
                    sdev = s1a.tile([8, 1], F32)
                    nc.scalar.activation(sdev[:], ssq[:], AF.Sqrt, bias=eps[:],
                                         scale=1.0 / QLR)
                    rinv = s1a.tile([8, 1], F32)
                    nc.vector.reciprocal(rinv[:], sdev[:])
                    cqn = s1a.tile([8, QLR], F32R)
                    nc.vector.tensor_scalar_mul(cqn[:], cqsum[:], rinv[:])
                    cqnT = s1b.tile([128, 12, 8], BF16)
                    with tc.tile_pool(name="s1tp", bufs=2, space="PSUM") as s1tp:
                        for k in range(12):
                            tp = s1tp.tile([128, 8], F32R, tag="tpr")
                            nc.tensor.transpose(tp[:], cqn[:, k * 128:(k + 1) * 128],
                                                idr[0:8, 0:8])
                            nc.vector.tensor_copy(cqnT[:, k, :], tp[:])

                # ===== Stage 2: q projections for 16 local heads =====
                qpe_sb = s1b.tile([8, HL, ROPE], BF16)
                qpe2 = s1b.tile([8, HL, ROPE], BF16)
                with tc.tile_pool(name="s2psA", bufs=1, space="PSUM") as s2psA:
                    qpe_ps = s2psA.tile([8, HL * ROPE], F32)
                    for kk in range(6):
                        wt = wqp.tile([128, 2, HL * ROPE], BF16, tag="wqr")
                        nc.sync.dma_start(
                            wt[:], wqr[:].rearrange("(kk p) n -> p kk n", p=128)
                            [:, 2 * kk:2 * kk + 2, :])
                        for j in range(2):
                            k = 2 * kk + j
                            for n in range(2):
                                nc.tensor.matmul(qpe_ps[:, n * 512:(n + 1) * 512],
                                                 cqnT[:, k, :],
                                                 wt[:, j, n * 512:(n + 1) * 512],
                                                 start=(k == 0), stop=(k == 11))
                    nc.vector.tensor_copy(
                        qpe_sb[:].rearrange("b h r -> b (h r)"), qpe_ps[:])
                # deinterleave q_pe (concat-halves permutation, matching host k)
                nc.vector.tensor_copy(qpe2[:, :, 0:32], qpe_sb[:, :, 0:ROPE:2])
                nc.vector.tensor_copy(qpe2[:, :, 32:64], qpe_sb[:, :, 1:ROPE:2])
                NQ2 = HL * KVLR // 2   # 4096
                HLH = HL // 2
                with tc.tile_pool(name="s2psB", bufs=1, space="PSUM") as s2psB:
                    for hf in range(2):
                        qps = s2psB.tile([8, NQ2], F32, tag="qn")
                        for k in range(12):
                            wt = wkp.tile([128, NQ2], BF16, tag="wuk")
                            nc.sync.dma_start(
                                wt[:], wuk[:].rearrange("(kk p) n -> p kk n", p=128)
                                [:, k, hf * NQ2:(hf + 1) * NQ2])
                            for n in range(NQ2 // 512):
                                nc.tensor.matmul(qps[:, n * 512:(n + 1) * 512],
                                                 cqnT[:, k, :],
                                                 wt[:, n * 512:(n + 1) * 512],
                                                 start=(k == 0), stop=(k == 11))
                        qn_h = s1b.tile([8, NQ2], BF16, tag="qnh")
                        nc.vector.tensor_copy(qn_h[:], qps[:])
                        nc.scalar.dma_start(
                            q_ag_in[:, hf * HLH:(hf + 1) * HLH, 0:KVLR],
                            qn_h[:].rearrange("b (h l) -> b h l", h=HLH))
                nc.scalar.dma_start(q_ag_in[:, :, KVLR:KVLR + ROPE], qpe2[:])
                coll("AllGather", mybir.AluOpType.bypass, q_ag_in, q_ag_out)

            # ===== Stage 3: attention over local kv shard, all 128 heads =====
            with (
                tc.tile_pool(name="s3", bufs=2) as s3,
                tc.tile_pool(name="s3o", bufs=4) as s3o,
                tc.tile_pool(name="scps", bufs=2, space="PSUM") as scps,
                tc.tile_pool(name="atps", bufs=2, space="PSUM") as atps,
                tc.tile_pool(name="s3tp", bufs=2, space="PSUM") as s3tp,
            ):
                for b in range(B):
                    ckv_b = ckv_tiles.pop(b)
                    qn_all = s3.tile([128, KVLR], BF16, tag="qn_all")
                    nc.scalar.dma_start(qn_all[:], q_ag_out[:, b, :, 0:KVLR])
                    qe_all = s3.tile([128, ROPE], BF16, tag="qe_all")
                    nc.scalar.dma_start(qe_all[:], q_ag_out[:, b, :, KVLR:KVLR + ROPE])
                    # transpose q
                    qnT = s3.tile([128, 4, 128], BF16, tag="qnT")
                    tp = s3tp.tile([128, 512], BF16, tag="tpb")
                    for lc in range(4):
                        nc.tensor.transpose(tp[:, lc * 128:(lc + 1) * 128],
                                            qn_all[:, lc * 128:(lc + 1) * 128], idb[:])
                    nc.vector.tensor_copy(qnT[:].rearrange("p a b -> p (a b)"), tp[:])
                    qeT = s3.tile([64, 128], BF16, tag="qeT")
                    tpq = s3tp.tile([64, 128], BF16, tag="tpb")
                    nc.tensor.transpose(tpq[:], qe_all[:], idb[:])
                    nc.vector.tensor_copy(qeT[:], tpq[:])
                    # transpose ckv -> [l=4x128, kv=KVL]
                    ckvT = s3.tile([128, 4, KVL], BF16, tag="ckvT")
                    for lc in range(4):
                        for g in range(KT // 4):
                            tp = s3tp.tile([128, 512], BF16, tag="tpb")
                            for j in range(4):
                                t = g * 4 + j
                                nc.tensor.transpose(tp[:, j * 128:(j + 1) * 128],
                                                    ckv_b[:, t, lc * 128:(lc + 1) * 128],
                                                    idb[:])
                            nc.vector.tensor_copy(ckvT[:, lc, g * 512:(g + 1) * 512], tp[:])
                    # scores = qn . ckv^T + qe . ke^T   [128h, KVL]
                    sc_ps = scps.tile([128, KVL], F32)
                    for t2 in range(KVL // 512):
                        sl = slice(t2 * 512, (t2 + 1) * 512)
                        for lc in range(4):
                            nc.tensor.matmul(sc_ps[:, sl], qnT[:, lc, :], ckvT[:, lc, sl],
                                             start=(lc == 0), stop=False)
                        nc.tensor.matmul(sc_ps[:, sl], qeT[:], ket_all[:, b, sl],
                                         start=False, stop=True)
                    # probs (unnormalized) + partial lsum
                    attn_sb = s3o.tile([128, KVLR + 1], BF16, tag="attn")
                    lsum = s3.tile([128, 1], F32, tag="lsum")
                    probs = s3.tile([128, KVL], BF16, tag="probs")
                    nc.scalar.activation(probs[:], sc_ps[:], AF.Exp, scale=SCALE,
                                         accum_out=lsum[:])
                    nc.vector.tensor_copy(attn_sb[:, KVLR:KVLR + 1], lsum[:])
                    # probs^T
                    probsT = s3.tile([128, KT, 128], BF16, tag="probsT")
                    for g in range(KT // 4):
                        tp = s3tp.tile([128, 512], BF16, tag="tpb")
                        for j in range(4):
                            t = g * 4 + j
                            nc.tensor.transpose(tp[:, j * 128:(j + 1) * 128],
                                                probs[:, t * 128:(t + 1) * 128], idb[:])
                        nc.vector.tensor_copy(
                            probsT[:, g * 4:(g + 1) * 4, :].rearrange("p a b -> p (a b)"),
                            tp[:])
                    # attn partial = probs^T . ckv  [128h, KVLR]
                    at_ps = atps.tile([128, KVLR], F32)
                    for t in range(KT):
                        nc.tensor.matmul(at_ps[:], probsT[:, t, :], ckv_b[:, t, :],
                                         start=(t == 0), stop=(t == KT - 1))
                    nc.vector.tensor_copy(attn_sb[:, 0:KVLR], at_ps[:])
                    # one store per batch on the otherwise-idle Pool queue:
                    # partitions (c,h) match dram rows (c,h)
                    nc.gpsimd.dma_start(at_rs_in[:, b, :, :], attn_sb[:])
                    if b + 3 < B:
                        ckv_tiles[b + 3] = prefetch_ckv(b + 3)
                coll("ReduceScatter", mybir.AluOpType.add, at_rs_in, at_rs_out)

            # ===== Stage 4: out = (attn/lsum) @ W_UV_O, head shard =====
            with (
                tc.tile_pool(name="s4", bufs=1) as s4,
                tc.tile_pool(name="oaps", bufs=1, space="PSUM") as oaps,
                tc.tile_pool(name="s4tp", bufs=2, space="PSUM") as s4tp,
            ):
                aT = s4.tile([128, HL * 4, 8], BF16)
                for hc in range(2):          # 8-head chunks to bound SBUF
                    o_sb = s4.tile([8, HL // 2, KVLR + 1], BF16, tag="osb")
                    nc.scalar.dma_start(
                        o_sb[:], at_rs_out[:, hc * (HL // 2):(hc + 1) * (HL // 2), :])
                    linv = s4.tile([8, HL // 2], F32, tag="linv")
                    nc.vector.reciprocal(linv[:], o_sb[:, :, KVLR])
                    for hh in range(HL // 2):
                        h = hc * (HL // 2) + hh
                        osc = s4.tile([8, KVLR], BF16, tag="osc")
                        nc.vector.tensor_scalar_mul(osc[:], o_sb[:, hh, 0:KVLR],
                                                    linv[:, hh:hh + 1])
                        tp = s4tp.tile([128, 32], BF16, tag="tpb4")
                        for lc in range(4):
                            nc.tensor.transpose(tp[:, lc * 8:(lc + 1) * 8],
                                                osc[:, lc * 128:(lc + 1) * 128],
                                                idb[0:8, 0:8])
                        nc.vector.tensor_copy(
                            aT[:, h * 4:(h + 1) * 4, :].rearrange("p a b -> p (a b)"),
                            tp[:])
                for half in range(2):
                    o_ps = oaps.tile([8, NHALF], F32, tag="ops")
                    for r2 in range(32):
                        wt = wvp.tile([128, 2, NHALF], BF16, tag="wuvo")
                        nc.sync.dma_start(
                            wt[:], wuvo[r2 * 256:(r2 + 1) * 256,
                                        half * NHALF:(half + 1) * NHALF]
                            .rearrange("(r p) n -> p r n", p=128))
                        for rr in range(2):
                            r = r2 * 2 + rr
                            for n5 in range(NHALF // 512):
                                nc.tensor.matmul(o_ps[:, n5 * 512:(n5 + 1) * 512],
                                                 aT[:, r, :],
                                                 wt[:, rr, n5 * 512:(n5 + 1) * 512],
                                                 start=(r == 0), stop=(r == 63))
                    outp = s4.tile([8, NHALF], F32, tag="outp")
                    nc.vector.tensor_copy(outp[:], o_ps[:])
                    nc.scalar.dma_start(o_ar_in[half][:], outp[:])
                    # first AR overlaps the second half's stream + matmuls
                    coll("AllReduce", mybir.AluOpType.add, o_ar_in[half], o_ar_out[half])
                    nc.scalar.dma_start(out[:, half * NHALF:(half + 1) * NHALF],
                                        o_ar_out[half][:])

    nc.compile()
    return nc


def make_in_maps(hidden_states, compressed_kv_normed_cache, k_pe_cache,
                 W_DQ, ln_w, W_QR, W_UQ_UK, W_UV_O):
    import ml_dtypes
    f32 = np.float32
    bf16 = ml_dtypes.bfloat16
    hidden_states = np.asarray(hidden_states, f32)
    ckv = np.asarray(compressed_kv_normed_cache, f32)
    kpe = np.asarray(k_pe_cache, f32)
    W_DQ = np.asarray(W_DQ, f32)
    ln_w = np.asarray(ln_w, f32)
    W_QR = (np.asarray(W_QR, f32) * ln_w[:, None]).astype(bf16)
    W_UQ_UK = (np.asarray(W_UQ_UK, f32) * ln_w[:, None]).astype(bf16)
    W_UV_O = np.asarray(W_UV_O, f32).astype(bf16)

    # host-side rope on k with relative positions (q stays un-roped), then
    # deinterleave to concat-halves and transpose to [ROPE, b, kv]
    inv = 1.0 / (THETA ** (np.arange(0, ROPE, 2, dtype=np.float64) / ROPE))
    rel = (np.arange(KV, dtype=np.float64) - (KV - 1))[:, None] * inv[None, :]
    cost = np.cos(rel).astype(f32)[None]  # [1, KV, 32]
    sint = np.sin(rel).astype(f32)[None]
    ev, od = kpe[:, :, 0::2], kpe[:, :, 1::2]
    ke = np.concatenate([ev * cost - od * sint, ev * sint + od * cost], axis=-1)
    keT = np.ascontiguousarray(ke.transpose(2, 0, 1)).astype(bf16)  # [64, B, KV]

    hsT = np.ascontiguousarray(hidden_states.T)  # [H, B]
    identr = np.eye(128, dtype=f32)
    identb = np.eye(128, dtype=f32).astype(bf16)
    ckv_b = ckv.astype(bf16)

    c = np.ascontiguousarray
    in_maps = []
    for ci in range(N_CORES):
        in_maps.append({
            "hst": c(hsT[ci * HD:(ci + 1) * HD, :]).astype(bf16),
            "wdq": c(W_DQ[ci * HD:(ci + 1) * HD, :]).astype(bf16),
            "wqr": c(W_QR[:, ci * HL * ROPE:(ci + 1) * HL * ROPE]),
            "wuk": c(W_UQ_UK[:, ci * HL * KVLR:(ci + 1) * HL * KVLR]),
            "ckv": c(ckv_b[:, ci * KVL:(ci + 1) * KVL, :]),
            "ket": c(keT[:, :, ci * KVL:(ci + 1) * KVL]),
            "identr": identr,
            "identb": identb,
            "wuvo": c(W_UV_O[ci * HL * KVLR:(ci + 1) * HL * KVLR, :]),
        })
    return in_maps


def kernel(**inputs) -> np.ndarray:
    from concourse import bass_utils

    if "nc" not in _CACHE:
        _CACHE["nc"] = build_nc()
    nc = _CACHE["nc"]
    in_maps = make_in_maps(**inputs)
    res = bass_utils.run_bass_kernel_spmd(nc, in_maps, core_ids=list(range(N_CORES)))
    return np.asarray(res.results[0]["out"], np.float32)

